# revision 6
# baseline (speedup 1.0000x reference)
"""Bilinear RoI pooling (7x7) on 8 Trainium2 NeuronCores.

Strategy (data-parallel over RoIs, per the sharding hint):
  - B=1024 boxes split into 8 slices of 128; the feature map is replicated.
  - Host builds a pair-interleaved INT8 copy of the zero-padded feature map:
    padded map P2 is (132,132,C) with a 2-px zero border; two row-pair copies
    E (rows 2e,2e+1) and O (rows 2o+1,2o+2) are stored as [pair, x, r, c] so
    the full 2x2 bilinear patch for any (y0,x0) corner is 4*C CONTIGUOUS int8
    bytes (2 adjacent 1KB slots) -> ONE 2KB gather window per (box, point).
  - int8 quantization is per-pixel (one scale over the 512 channels of each
    (y,x)); the dequant scale of each corner pixel is folded into that
    corner's bilinear weight on the host, so the device blends raw int8
    corners with premultiplied f32 scalars -> no device-side dequant work.
    Host work stays O(B*49) scalars + one O(H*W*C) repack/quantize pass (the
    same spirit as the fp16 repack this replaces); the O(B*49*C) gather+blend
    stays on device.
  - Per core the 49x128 gathers are issued as 7 dma_gather ops (7 grid points
    x 128 boxes = 896 indices each): SWDGE emits one descriptor per index, so
    per-op overhead is amortized ~128x vs per-(box,point) indirect DMAs.
    Gather indices are int16, wrapped into 16 partitions and REPLICATED in
    all 8 16-partition groups (each GPSIMD Q7 core reads its own copy).
  - Blend per point: 2 muls on ACT (int8 in, fp16 out), on DVE one
    tensor_scalar + one fused scalar_tensor_tensor (int8 in) + 2 fp16 adds.
    Both engines stay under the ~54us HBM roofline of the int8 gathers.
  - Out-of-bounds corners read zero border rows/cols (clamped indices), so no
    in-bounds masking is needed.

Device layout: partition = box (128/core); 49 grid points along free dim.
Output is fp16 on device; host casts to f32.
"""

import numpy as np

P = 128          # boxes per core == SBUF partitions
C = 512          # channels
NPT = 49         # 7*7 grid points
WP2 = 132        # padded width (2 zero cols each side)
HP2 = 132        # padded height (2 zero rows top, 2 bottom)
NBLK_E = 66      # even row-pairs (rows 0..131)
NBLK_O = 65      # odd row-pairs (rows 1..130)
NSLOT_E = NBLK_E * WP2
NSLOT = (NBLK_E + NBLK_O) * WP2   # 17292 slots of [2, C] int8 = 1KB
NCORES = 8
CHUNK = 7        # grid points per dma_gather op
NCHUNK = NPT // CHUNK
IDXCOLS = CHUNK * P // 16         # idx cols per chunk in the wrapped layout

_STATE = {}


def _build_nc(repeats=1, gbufs=4, abufs=4, tbufs=8, mode="full"):
    """mode: 'full' | 'noblend' (gather+copy+store) | 'nodma' (blend from
    const tile, no gathers)."""
    import concourse.bass as bass
    import concourse.bacc as bacc
    import concourse.tile as tile
    from concourse import mybir

    F32 = mybir.dt.float32
    F16 = mybir.dt.float16
    I16 = mybir.dt.int16
    I8 = mybir.dt.int8
    Alu = mybir.AluOpType

    nc = bacc.Bacc()
    fgat = nc.declare_dram_parameter("fgat", [NSLOT, 2 * C], I8, isOutput=False)
    gidx = nc.declare_dram_parameter(
        "gidx", [P, NCHUNK * IDXCOLS], I16, isOutput=False)
    # wts = [w00*s00 | w10*s10 | w01*s01 | w11*s11] blocks of NPT, f32
    wts = nc.declare_dram_parameter("wts", [P, 4 * NPT], F32, isOutput=False)
    out = nc.declare_dram_parameter("out", [P, NPT * C], F16, isOutput=True)

    with tile.TileContext(nc) as tc:
        with (
            tc.tile_pool(name="const", bufs=1) as cpool,
            tc.tile_pool(name="apool", bufs=abufs) as apool,
            tc.tile_pool(name="gpool", bufs=gbufs) as gpool,
            tc.tile_pool(name="tpool", bufs=tbufs) as tpool,
        ):
            idx = cpool.tile([P, NCHUNK * IDXCOLS], I16)
            nc.sync.dma_start(out=idx[:], in_=gidx[:])
            w = cpool.tile([P, 4 * NPT], F32)
            nc.scalar.dma_start(out=w[:], in_=wts[:])
            w00 = w[:, 0 * NPT:1 * NPT]
            w10 = w[:, 1 * NPT:2 * NPT]
            w01 = w[:, 2 * NPT:3 * NPT]
            w11 = w[:, 3 * NPT:4 * NPT]

            gconst = None
            if mode == "nodma":
                gconst = cpool.tile([P, CHUNK * 4 * C], I8, tag="gconst")
                nc.vector.memset(gconst[:], 3)

            for rep in range(repeats):
                for ch in range(NCHUNK):
                    if mode == "nodma":
                        g = gconst
                    else:
                        g = gpool.tile([P, CHUNK * 4 * C], I8, tag="g")
                        # overlapping 2KB windows at 1KB slot granularity
                        src = fgat[:]
                        src.ap[0] = [2 * C, NSLOT - 1]
                        src.ap[1] = [1, 4 * C]
                        g3 = g[:].rearrange(
                            "p (a b) -> p a b", a=CHUNK, b=4 * C)
                        nc.gpsimd.dma_gather(
                            out_ap=g3,
                            in_ap=src,
                            idxs_ap=idx[:, ch * IDXCOLS:(ch + 1) * IDXCOLS],
                            num_idxs=CHUNK * P,
                            num_idxs_reg=CHUNK * P,
                            elem_size=4 * C,
                            elem_step=2 * C,
                        )
                    afat = apool.tile([P, CHUNK * C], F16, tag="afat")
                    for k in range(CHUNK):
                        t = ch * CHUNK + k
                        A0 = g[:, (4 * k + 0) * C:(4 * k + 1) * C]
                        B0 = g[:, (4 * k + 1) * C:(4 * k + 2) * C]
                        A1 = g[:, (4 * k + 2) * C:(4 * k + 3) * C]
                        B1 = g[:, (4 * k + 3) * C:(4 * k + 4) * C]
                        ac = afat[:, k * C:(k + 1) * C]
                        if mode == "noblend":
                            nc.vector.tensor_copy(out=ac, in_=A0)
                            continue
                        u1 = tpool.tile([P, C], F16, tag="u1")
                        nc.scalar.mul(u1[:], B0, w10[:, t:t + 1])
                        u2 = tpool.tile([P, C], F16, tag="u2")
                        nc.scalar.mul(u2[:], B1, w11[:, t:t + 1])
                        t1 = tpool.tile([P, C], F16, tag="t1")
                        nc.vector.tensor_scalar(
                            out=t1[:], in0=A0, scalar1=w00[:, t:t + 1],
                            scalar2=None, op0=Alu.mult)
                        s1 = tpool.tile([P, C], F16, tag="s1")
                        nc.vector.scalar_tensor_tensor(
                            out=s1[:], in0=A1, scalar=w01[:, t:t + 1],
                            in1=t1[:], op0=Alu.mult, op1=Alu.add)
                        u3 = tpool.tile([P, C], F16, tag="u3")
                        nc.vector.tensor_tensor(
                            out=u3[:], in0=u1[:], in1=u2[:], op=Alu.add)
                        nc.vector.tensor_tensor(
                            out=ac, in0=s1[:], in1=u3[:], op=Alu.add)
                    nc.sync.dma_start(
                        out=out[:, ch * CHUNK * C:(ch + 1) * CHUNK * C],
                        in_=afat[:, 0:CHUNK * C])

    nc.compile()
    return nc


def _prep_fgat(features):
    """Pair-interleaved int8 gather map + per-pixel dequant scales.

    Returns (fgat int8 [NSLOT, C], S f32 [HP2, WP2]) where
    quantized(y, x, c) = round(p2[y, x, c] / S[y, x]).
    """
    f = np.asarray(features, dtype=np.float32)
    p2 = np.zeros((HP2, WP2, C), dtype=np.float32)
    p2[2:130, 2:130, :] = f
    s = np.max(np.abs(p2), axis=2) / 127.0
    s[s == 0.0] = 1.0
    q = np.rint(p2 / s[:, :, None]).astype(np.int8)
    # E[e, x, r, c] = q[2e+r, x, c]; O[o, x, r, c] = q[2o+1+r, x, c]
    e = np.ascontiguousarray(
        q.reshape(NBLK_E, 2, WP2, C).transpose(0, 2, 1, 3)
    ).reshape(NSLOT_E, 2 * C)
    o = np.ascontiguousarray(
        q[1:131].reshape(NBLK_O, 2, WP2, C).transpose(0, 2, 1, 3)
    ).reshape(NBLK_O * WP2, 2 * C)
    return np.concatenate([e, o], axis=0), s.astype(np.float32)


def _prep_meta(boxes, s):
    """Per-(box,point) gather slot index and scale-folded corner weights.

    Mirrors the reference affine-grid math in float32:
      yf = BY*(0.5*bh-0.5) + (yc-1),  xf = BX*(0.5*bw-0.5) + (xc-1)
    with BY/BX the 7x7 [-1,1] grid; then y0=floor(yf), wy=yf-y0 (same for x).
    OOB corners are mapped to zero border rows/cols of the padded map, so the
    weights need no in-bounds masking. Each corner weight is multiplied by
    that corner pixel's int8 dequant scale.

    Returns (slot int32 [B,49], wts f32 [B, 4*49]).
    """
    b = np.asarray(boxes, dtype=np.float32)
    xc, yc, bw, bh = b[:, 0:1], b[:, 1:2], b[:, 2:3], b[:, 3:4]
    base = np.linspace(-1.0, 1.0, 7).astype(np.float32)
    BY = np.repeat(base, 7)[None, :]   # (1,49)
    BX = np.tile(base, 7)[None, :]
    yf = (BY * (np.float32(0.5) * bh - np.float32(0.5)) + (yc - 1)).astype(np.float32)
    xf = (BX * (np.float32(0.5) * bw - np.float32(0.5)) + (xc - 1)).astype(np.float32)
    y0 = np.floor(yf)
    x0 = np.floor(xf)
    wy = yf - y0
    wx = xf - x0
    wyc = np.float32(1.0) - wy
    wxc = np.float32(1.0) - wx
    # padded coords of the gathered 2x2 patch: rows pyA, pyA+1; cols px, px+1
    pyA = np.clip(y0 + 2.0, 0.0, 130.0)
    px = np.clip(x0, -2.0, 128.0) + 2.0
    half = np.floor(pyA * 0.5)
    par = pyA - 2.0 * half
    slot = (par * NSLOT_E + half * WP2 + px).astype(np.int32)
    ri = pyA.astype(np.int32)
    ci = px.astype(np.int32)
    s00 = s[ri, ci]
    s10 = s[ri + 1, ci]
    s01 = s[ri, ci + 1]
    s11 = s[ri + 1, ci + 1]
    # weights for gathered layout [A0, B0, A1, B1]
    wts = np.concatenate(
        [wyc * wxc * s00, wy * wxc * s10, wyc * wx * s01, wy * wx * s11],
        axis=1).astype(np.float32)
    return slot, wts


def _wrap_idx(slot_core):
    """[P, 49] int32 slots -> [P, NCHUNK*IDXCOLS] int16 wrapped+replicated."""
    gi = np.zeros((P, NCHUNK * IDXCOLS), dtype=np.int16)
    for ch in range(NCHUNK):
        flat = slot_core[:, ch * CHUNK:(ch + 1) * CHUNK].T.reshape(-1)  # j*P+p
        cols = flat.reshape(IDXCOLS, 16)  # i//16 , i%16
        for k in range(8):
            gi[16 * k:16 * (k + 1), ch * IDXCOLS:(ch + 1) * IDXCOLS] = \
                cols.T.astype(np.int16)
    return gi


def _in_maps(features, boxes):
    fgat, s = _prep_fgat(features)
    slot, wts = _prep_meta(boxes, s)
    maps = []
    for k in range(NCORES):
        sl = slot[k * P:(k + 1) * P]
        maps.append({
            "fgat": fgat,
            "gidx": _wrap_idx(sl),
            "wts": np.ascontiguousarray(wts[k * P:(k + 1) * P]),
        })
    return maps


def kernel(features, boxes, image_height=128, image_width=128):
    from concourse.bass_utils import run_bass_kernel_spmd

    if "nc" not in _STATE:
        _STATE["nc"] = _build_nc()
    nc = _STATE["nc"]

    in_maps = _in_maps(features, boxes)
    res = run_bass_kernel_spmd(
        nc, in_maps, core_ids=list(range(NCORES)),
        trace=_STATE.get("trace", False),
    )
    _STATE["last"] = res
    out = np.concatenate(
        [res.results[k]["out"].reshape(P, 7, 7, C).astype(np.float32)
         for k in range(NCORES)],
        axis=0,
    )
    return out


# revision 17
# speedup vs baseline: 1.2182x; 1.2182x over previous
"""Bilinear RoI pooling (7x7) on 8 Trainium2 NeuronCores.

Strategy (data-parallel over RoIs, per the sharding hint):
  - B=1024 boxes split into 8 slices of 128; the feature map is replicated.
  - Host builds TWO pair-interleaved copies of the zero-padded feature map
    (132x132 with a 2-px zero border), indexed by the same slot numbering:
    each slot is one (row-pair, x) column; the 2x2 bilinear patch of any
    corner is 2 adjacent slots = one contiguous gather window.
      * fgat16: fp16, 2KB slots (4KB windows)
      * fgat8:  int8 quantized per-PIXEL (scale folded into that corner's
        bilinear weight on the host -> no device dequant), 1KB slots
  - 28 of the 49 grid points gather from the int8 table, 21 from the fp16
    table: this balances the HBM roofline (~70us) against the ACT-engine
    int8-multiply rate and keeps the DVE duty-cycle low enough that SWDGE
    descriptor generation (which DVE 2-port mode locks out of SBUF) never
    starves.  Pure-int8 variants measure SLOWER than fp16 despite halving
    DMA bytes because both ALU engines read int8 at 1.7-4x lower rates.
  - Gathers are issued as dma_gather ops (7 grid points x 128 boxes = 896
    indices each): SWDGE emits one descriptor per index, amortizing per-op
    overhead ~128x vs per-(box,point) indirect DMAs.  Gather indices are
    int16, wrapped into 16 partitions and REPLICATED in all 8 16-partition
    groups (each GPSIMD Q7 core reads its own copy).
  - Blend: int8 points run 3 muls on ACT + 1 fused mul-add and 2 adds on
    DVE; fp16 points run a 4-op fused DVE chain.  int8/fp16 chunks are
    interleaved so ACT and DVE fill in parallel.
  - Out-of-bounds corners read zero border rows/cols (clamped indices), so no
    in-bounds masking is needed.

Device layout: partition = box (128/core); 49 grid points along free dim.
Output is fp16 on device; host casts to f32.
"""

import numpy as np

P = 128          # boxes per core == SBUF partitions
C = 512          # channels
NPT = 49         # 7*7 grid points
WP2 = 132        # padded width (2 zero cols each side)
HP2 = 132        # padded height (2 zero rows top, 2 bottom)
NBLK_E = 66      # even row-pairs (rows 0..131)
NBLK_O = 65      # odd row-pairs (rows 1..130)
NSLOT_E = NBLK_E * WP2
NSLOT = (NBLK_E + NBLK_O) * WP2   # 17292 slots
NCORES = 8
CHUNK = 7        # grid points per dma_gather op
NCHUNK = NPT // CHUNK
NCH8 = 4         # int8 chunks (points 0..27)
NPT8 = NCH8 * CHUNK
IDXCOLS = CHUNK * P // 16         # idx cols per chunk in the wrapped layout

_STATE = {}


def _build_nc(repeats=1, g8bufs=3, g16bufs=2, abufs=4, tbufs=8, mode="full",
              nch8=NCH8):
    """mode: 'full' | 'noblend' (gather+copy+store) | 'nodma' (blend from
    const tiles, no gathers)."""
    import concourse.bass as bass
    import concourse.bacc as bacc
    import concourse.tile as tile
    from concourse import mybir

    F32 = mybir.dt.float32
    F16 = mybir.dt.float16
    I16 = mybir.dt.int16
    I8 = mybir.dt.int8
    Alu = mybir.AluOpType

    nc = bacc.Bacc()
    fgat8 = nc.declare_dram_parameter("fgat8", [NSLOT, 2 * C], I8, isOutput=False)
    fgat16 = nc.declare_dram_parameter("fgat16", [NSLOT, 2 * C], F16, isOutput=False)
    gidx = nc.declare_dram_parameter(
        "gidx", [P, NCHUNK * IDXCOLS], I16, isOutput=False)
    # wts = [w00 | w10 | w01 | w11] blocks of NPT, f32 (scale-folded for the
    # int8 points)
    wts = nc.declare_dram_parameter("wts", [P, 4 * NPT], F32, isOutput=False)
    out = nc.declare_dram_parameter("out", [P, NPT * C], F16, isOutput=True)

    with tile.TileContext(nc) as tc:
        with (
            tc.tile_pool(name="const", bufs=1) as cpool,
            tc.tile_pool(name="apool", bufs=abufs) as apool,
            tc.tile_pool(name="g8pool", bufs=g8bufs) as g8pool,
            tc.tile_pool(name="g16pool", bufs=g16bufs) as g16pool,
            tc.tile_pool(name="tpool", bufs=tbufs) as tpool,
        ):
            idx = cpool.tile([P, NCHUNK * IDXCOLS], I16)
            nc.sync.dma_start(out=idx[:], in_=gidx[:])
            w = cpool.tile([P, 4 * NPT], F32)
            nc.scalar.dma_start(out=w[:], in_=wts[:])
            w00 = w[:, 0 * NPT:1 * NPT]
            w10 = w[:, 1 * NPT:2 * NPT]
            w01 = w[:, 2 * NPT:3 * NPT]
            w11 = w[:, 3 * NPT:4 * NPT]

            gc8 = gc16 = None
            if mode == "nodma":
                gc8 = cpool.tile([P, CHUNK * 4 * C], I8, tag="gc8")
                nc.vector.memset(gc8[:], 3)
                gc16 = cpool.tile([P, CHUNK * 4 * C], F16, tag="gc16")
                nc.vector.memset(gc16[:], 1.0)

            # interleave int8/fp16 chunks so ACT (int8 muls) and DVE (fp16
            # chain) fill in parallel
            order = []
            a, b = 0, nch8
            while a < nch8 or b < NCHUNK:
                if a < nch8:
                    order.append(a); a += 1
                if b < NCHUNK:
                    order.append(b); b += 1

            def gather(ch, is8):
                pool, tabl, dt, esz = (
                    (g8pool, fgat8, I8, 4 * C) if is8
                    else (g16pool, fgat16, F16, 4 * C))
                g = pool.tile([P, CHUNK * 4 * C], dt, tag="g")
                src = tabl[:]
                src.ap[0] = [2 * C, NSLOT - 1]
                src.ap[1] = [1, 4 * C]
                g3 = g[:].rearrange("p (a b) -> p a b", a=CHUNK, b=4 * C)
                nc.gpsimd.dma_gather(
                    out_ap=g3,
                    in_ap=src,
                    idxs_ap=idx[:, ch * IDXCOLS:(ch + 1) * IDXCOLS],
                    num_idxs=CHUNK * P,
                    num_idxs_reg=CHUNK * P,
                    elem_size=esz,
                    elem_step=2 * C,
                )
                return g

            for rep in range(repeats):
                for ch in order:
                    is8 = ch < nch8
                    if mode == "nodma":
                        g = gc8 if is8 else gc16
                    else:
                        g = gather(ch, is8)
                    afat = apool.tile([P, CHUNK * C], F16, tag="afat")
                    for k in range(CHUNK):
                        t = ch * CHUNK + k
                        A0 = g[:, (4 * k + 0) * C:(4 * k + 1) * C]
                        B0 = g[:, (4 * k + 1) * C:(4 * k + 2) * C]
                        A1 = g[:, (4 * k + 2) * C:(4 * k + 3) * C]
                        B1 = g[:, (4 * k + 3) * C:(4 * k + 4) * C]
                        ac = afat[:, k * C:(k + 1) * C]
                        if mode == "noblend":
                            nc.vector.tensor_copy(out=ac, in_=A0)
                            continue
                        if is8:
                            u1 = tpool.tile([P, C], F16, tag="u1")
                            nc.scalar.mul(u1[:], B0, w10[:, t:t + 1])
                            u2 = tpool.tile([P, C], F16, tag="u2")
                            nc.scalar.mul(u2[:], A1, w01[:, t:t + 1])
                            u3 = tpool.tile([P, C], F16, tag="u3")
                            nc.scalar.mul(u3[:], B1, w11[:, t:t + 1])
                            s1 = tpool.tile([P, C], F16, tag="s1")
                            nc.vector.scalar_tensor_tensor(
                                out=s1[:], in0=A0, scalar=w00[:, t:t + 1],
                                in1=u1[:], op0=Alu.mult, op1=Alu.add)
                            s2 = tpool.tile([P, C], F16, tag="s2")
                            nc.vector.tensor_tensor(
                                out=s2[:], in0=u2[:], in1=u3[:], op=Alu.add)
                            nc.vector.tensor_tensor(
                                out=ac, in0=s1[:], in1=s2[:], op=Alu.add)
                        else:
                            nc.vector.tensor_scalar(
                                out=ac, in0=A0, scalar1=w00[:, t:t + 1],
                                scalar2=None, op0=Alu.mult)
                            nc.vector.scalar_tensor_tensor(
                                out=ac, in0=B0, scalar=w10[:, t:t + 1],
                                in1=ac, op0=Alu.mult, op1=Alu.add)
                            nc.vector.scalar_tensor_tensor(
                                out=ac, in0=A1, scalar=w01[:, t:t + 1],
                                in1=ac, op0=Alu.mult, op1=Alu.add)
                            nc.vector.scalar_tensor_tensor(
                                out=ac, in0=B1, scalar=w11[:, t:t + 1],
                                in1=ac, op0=Alu.mult, op1=Alu.add)
                    nc.sync.dma_start(
                        out=out[:, ch * CHUNK * C:(ch + 1) * CHUNK * C],
                        in_=afat[:, 0:CHUNK * C])

    nc.compile()
    return nc


def _prep_fgat(features):
    """Pair-interleaved fp16 + per-pixel-quantized int8 gather maps.

    Slot s covers rows (y0, y0+1) of one padded column x:
      fgat16[s] = [row0 fp16 (1KB... 512 vals) | row1 fp16]
      fgat8[s]  = [row0 int8 (512B) | row1 int8], row r quantized by
                  S[y, x] = maxabs(p2[y, x, :]) / 127 (0 -> 1)
    Returns (fgat8 int8 [NSLOT, 2C], fgat16 fp16 [NSLOT, 2C], S [HP2, WP2]).
    """
    f = np.asarray(features, dtype=np.float32)
    p2 = np.zeros((HP2, WP2, C), dtype=np.float32)
    p2[2:130, 2:130, :] = f
    s = np.max(np.abs(p2), axis=2) / 127.0
    s[s == 0.0] = 1.0
    q = np.rint(p2 / s[:, :, None]).astype(np.int8)
    p16 = p2.astype(np.float16)

    def interleave(a):
        e = np.ascontiguousarray(
            a.reshape(NBLK_E, 2, WP2, C).transpose(0, 2, 1, 3)
        ).reshape(NSLOT_E, 2 * C)
        o = np.ascontiguousarray(
            a[1:131].reshape(NBLK_O, 2, WP2, C).transpose(0, 2, 1, 3)
        ).reshape(NBLK_O * WP2, 2 * C)
        return np.concatenate([e, o], axis=0)

    return interleave(q), interleave(p16), s.astype(np.float32)


def _prep_meta(boxes, s, npt8=NPT8):
    """Per-(box,point) gather slot index and blend weights.

    Mirrors the reference affine-grid math in float32:
      yf = BY*(0.5*bh-0.5) + (yc-1),  xf = BX*(0.5*bw-0.5) + (xc-1)
    with BY/BX the 7x7 [-1,1] grid; then y0=floor(yf), wy=yf-y0 (same for x).
    OOB corners are mapped to zero border rows/cols of the padded map, so the
    weights need no in-bounds masking.  For the first npt8 points (int8
    gathers) each corner weight is multiplied by that corner pixel's int8
    dequant scale.

    Returns (slot int32 [B,49], wts f32 [B, 4*49]).
    """
    b = np.asarray(boxes, dtype=np.float32)
    xc, yc, bw, bh = b[:, 0:1], b[:, 1:2], b[:, 2:3], b[:, 3:4]
    base = np.linspace(-1.0, 1.0, 7).astype(np.float32)
    BY = np.repeat(base, 7)[None, :]   # (1,49)
    BX = np.tile(base, 7)[None, :]
    yf = (BY * (np.float32(0.5) * bh - np.float32(0.5)) + (yc - 1)).astype(np.float32)
    xf = (BX * (np.float32(0.5) * bw - np.float32(0.5)) + (xc - 1)).astype(np.float32)
    y0 = np.floor(yf)
    x0 = np.floor(xf)
    wy = yf - y0
    wx = xf - x0
    wyc = np.float32(1.0) - wy
    wxc = np.float32(1.0) - wx
    pyA = np.clip(y0 + 2.0, 0.0, 130.0)
    px = np.clip(x0, -2.0, 128.0) + 2.0
    half = np.floor(pyA * 0.5)
    par = pyA - 2.0 * half
    slot = (par * NSLOT_E + half * WP2 + px).astype(np.int32)
    ri = pyA.astype(np.int32)
    ci = px.astype(np.int32)
    is8 = (np.arange(NPT) < npt8)[None, :]
    s00 = np.where(is8, s[ri, ci], np.float32(1.0))
    s10 = np.where(is8, s[ri + 1, ci], np.float32(1.0))
    s01 = np.where(is8, s[ri, ci + 1], np.float32(1.0))
    s11 = np.where(is8, s[ri + 1, ci + 1], np.float32(1.0))
    wts = np.concatenate(
        [wyc * wxc * s00, wy * wxc * s10, wyc * wx * s01, wy * wx * s11],
        axis=1).astype(np.float32)
    return slot, wts


def _wrap_idx(slot_core):
    """[P, 49] int32 slots -> [P, NCHUNK*IDXCOLS] int16 wrapped+replicated."""
    gi = np.zeros((P, NCHUNK * IDXCOLS), dtype=np.int16)
    for ch in range(NCHUNK):
        flat = slot_core[:, ch * CHUNK:(ch + 1) * CHUNK].T.reshape(-1)  # j*P+p
        cols = flat.reshape(IDXCOLS, 16)  # i//16 , i%16
        for k in range(8):
            gi[16 * k:16 * (k + 1), ch * IDXCOLS:(ch + 1) * IDXCOLS] = \
                cols.T.astype(np.int16)
    return gi


def _in_maps(features, boxes):
    fgat8, fgat16, s = _prep_fgat(features)
    slot, wts = _prep_meta(boxes, s)
    maps = []
    for k in range(NCORES):
        sl = slot[k * P:(k + 1) * P]
        maps.append({
            "fgat8": fgat8,
            "fgat16": fgat16,
            "gidx": _wrap_idx(sl),
            "wts": np.ascontiguousarray(wts[k * P:(k + 1) * P]),
        })
    return maps


def kernel(features, boxes, image_height=128, image_width=128):
    from concourse.bass_utils import run_bass_kernel_spmd

    if "nc" not in _STATE:
        _STATE["nc"] = _build_nc()
    nc = _STATE["nc"]

    in_maps = _in_maps(features, boxes)
    res = run_bass_kernel_spmd(
        nc, in_maps, core_ids=list(range(NCORES)),
        trace=_STATE.get("trace", False),
    )
    _STATE["last"] = res
    out = np.concatenate(
        [res.results[k]["out"].reshape(P, 7, 7, C).astype(np.float32)
         for k in range(NCORES)],
        axis=0,
    )
    return out
